# revision 1
# baseline (speedup 1.0000x reference)
"""Causal self-attention kernel for 8 Trainium2 NeuronCores.

Problem: B=2, T=2048, D=1024, 16 heads x 64. Tensor-parallel over heads:
core c owns heads [2c, 2c+1] (128 projection columns), computes its partial
output through wo's matching 128 rows; host sums the 8 partials (TP
all-reduce done at unshard time).

Device-side dataflow per core (all layouts chosen so NO on-device
transposes of activations or probabilities are needed):
  - x is pre-transposed on host to xT [D, B*T] so the contraction dim is
    always the SBUF partition dim.
  - Q^T, K^T, V^T projections: stationary = weight k-tile [128,128],
    moving = xT chunk -> out [128 (2 heads x 64), t] in PSUM.
  - V^T is transposed 128x128-wise on the PE (identity matmul) into
    V_ext tiles [s=128, 65] per head whose 65th column is ones: the
    attention AV matmul then yields both O^T and the softmax denominator.
  - scores: stationary = K^T s-tile [64,128], moving = Q^T t-chunk [64,512]
    -> S^T [s=128, t=512] in PSUM; exp on the scalar engine; causal mask by
    elementwise multiply with a host-provided 0/1 tile (only on the 4
    diagonal-crossing s-tiles of each t-chunk); fully-masked chunks skipped.
  - AV: stationary = V_ext s-tile [128,65], moving = E^T chunk [128,512]
    -> accumulate O^T_ext [65, t] over s-tiles in PSUM.
  - normalize: reciprocal of denom row, PE-broadcast (ones-column matmul)
    across 64 partitions, one DVE multiply.
  - out projection: stationary = O^T t-tile [64,128], moving = wo rows
    [64,512]; the two heads accumulate in PSUM; copied to SBUF and DMAd out.
Matmuls run as float32r (full-rate single-pass fp32) with fp32 PSUM.
"""

import sys

if "/opt/trn_rl_repo" not in sys.path:
    sys.path.insert(0, "/opt/trn_rl_repo")

import numpy as np

N_HEAD = 16
D_HEAD = 64
D = 1024
B = 2
T = 2048
NCORES = 8
HPC = N_HEAD // NCORES          # heads per core = 2
DC = HPC * D_HEAD               # projection cols per core = 128
BT = B * T                      # 4096
TCH = 512                       # t-chunk (PSUM bank = 512 fp32)
NTCH = T // TCH                 # 4 t-chunks per batch
NST = T // 128                  # 16 s-tiles per batch

_cache = {}


def _build(reps=1):
    import concourse.bass as bass
    import concourse.mybir as mybir
    import concourse.tile as tile
    from concourse.masks import make_identity

    f32 = mybir.dt.float32
    f32r = mybir.dt.float32r
    Exp = mybir.ActivationFunctionType.Exp
    mult = mybir.AluOpType.mult

    nc = bass.Bass("TRN2", target_bir_lowering=False, debug=False)

    xT_d = nc.dram_tensor("xT", [D, BT], f32r, kind="ExternalInput").ap()
    wq_d = nc.dram_tensor("wq", [D, DC], f32r, kind="ExternalInput").ap()
    wk_d = nc.dram_tensor("wk", [D, DC], f32r, kind="ExternalInput").ap()
    wv_d = nc.dram_tensor("wv", [D, DC], f32r, kind="ExternalInput").ap()
    wo_d = nc.dram_tensor("wo", [DC, D], f32r, kind="ExternalInput").ap()
    mk_d = nc.dram_tensor("masks", [128, 128], f32, kind="ExternalInput").ap()
    out_d = nc.dram_tensor("out", [BT, D], f32, kind="ExternalOutput").ap()

    def _split_matmul_waits():
        """walrus codegen allows a single sync-wait on several instruction
        encodings (self-loading fp32r matmul LW structs, DMA triggers);
        hoist extra waits onto same-engine NoOps placed just before."""
        for bb in nc.m.functions[0].blocks:
            out = []
            for ins in bb.instructions:
                si = getattr(ins, "sync_info", None)
                if si and len(si.on_wait) > 1:
                    for i, w in enumerate(list(si.on_wait[:-1])):
                        nop = mybir.InstNoOp(
                            name=f"{ins.name}-ws{i}",
                            engine=ins.engine,
                            sync_info=mybir.SyncInfo(on_wait=[w], on_update=[]),
                        )
                        nc.register_instruction(nop)
                        out.append(nop)
                    si.on_wait = [si.on_wait[-1]]
                out.append(ins)
            bb.instructions = out

    with tile.TileContext(nc) as tc:
        with (
            tc.tile_pool(name="const", bufs=1) as constp,
            tc.tile_pool(name="wpool", bufs=1) as wpool,
            tc.tile_pool(name="resid", bufs=1) as resid,
            tc.tile_pool(name="xs", bufs=2) as xsp,
            tc.tile_pool(name="et", bufs=6) as etp,
            tc.tile_pool(name="oT", bufs=2) as oTp,
            tc.tile_pool(name="dn", bufs=2) as dnp,
            tc.tile_pool(name="rb", bufs=2) as rbp,
            tc.tile_pool(name="ostg", bufs=3) as ostgp,
        ):
            ident_f = constp.tile([128, 128], f32)
            make_identity(nc, ident_f[:])
            ident = constp.tile([128, 128], f32r)
            nc.vector.tensor_copy(out=ident[:], in_=ident_f[:])
            bf16 = mybir.dt.bfloat16
            ident_b = constp.tile([128, 128], bf16)
            nc.vector.tensor_copy(out=ident_b[:], in_=ident_f[:])
            mask_b = constp.tile([128, 128], bf16)
            # denom-broadcast selector rows (lane-matched to dn2):
            #   row 64 (head0's denom partition): [1]*64 + [0]*64
            #   row 0  (head1's denom partition): [0]*64 + [1]*64
            # K=1 matmuls with these as stationary spread each denom row
            # onto its head's 64 output partitions.
            sel_f = constp.tile([65, 128], f32)
            nc.vector.memset(sel_f[:], 0.0)
            nc.vector.memset(sel_f[64:65, 0:64], 1.0)
            nc.vector.memset(sel_f[0:1, 64:128], 1.0)
            sel_t = constp.tile([65, 128], f32r)
            nc.vector.tensor_copy(out=sel_t[:], in_=sel_f[:])

            # weights: wq_s[p, kk, n] = wq[kk*128+p, n]
            # qkv weights on the sync hwdge queue (critical path to first
            # matmul); masks/wo on the scalar queue.
            wq_s = wpool.tile([128, 8, 128], f32r, tag="wq")
            wk_s = wpool.tile([128, 8, 128], f32r, tag="wk")
            wv_s = wpool.tile([128, 8, 128], f32r, tag="wv")
            nc.sync.dma_start(wq_s[:], wq_d.rearrange("(a p) n -> p a n", p=128))
            mask_f = constp.tile([128, 128], f32)
            # wo natural layout [128, 1024]: partitions = both heads' d rows
            wo2_s = wpool.tile([DC, D], f32r, tag="wo")

            qT = resid.tile([128, BT], f32r, tag="qT")
            kT = resid.tile([128, BT], f32r, tag="kT")
            vT = resid.tile([128, BT], f32r, tag="vT")
            # Per s-tile layout (width 193):
            #   cols   0:64  V_h0            -> head0 AV lhsT = cols 0:65
            #   col      64  ones (h0 denom)    (O_h0 at psum parts 0:64,
            #   col      65  ones (h1 denom)     denom at part 64)
            #   cols 66:129  zeros           -> head1 AV lhsT = cols 65:193
            #   cols 129:193 V_h1               (denom at psum part 0, O_h1
            #                                    at psum parts 64:128)
            # Head outputs land directly on their final partitions: DVE
            # lanes cannot cross partitions, so psum->sbuf copies must be
            # partition-aligned.
            v_ext = resid.tile([128, 2 * NST, 193], f32r, tag="vext")

            # ---- stage A: QKV^T projections over 8 t-chunks of 512,
            # with V 128x128 PE-transposes fused in per chunk ----
            # memset can't encode f32r: build the [1,1,0...] middle-column
            # pattern in f32 and round-copy it into each s-tile's cols 64:129
            pat_f = constp.tile([128, 65], f32)
            nc.vector.memset(pat_f[:], 0.0)
            nc.vector.memset(pat_f[:, 0:2], 1.0)
            for _rep in range(reps):
                with (
                    tc.tile_pool(name="qkvps", bufs=2, space="PSUM") as qkvps,
                    tc.tile_pool(name="trps", bufs=2, space="PSUM") as trps,
                ):
                    for tcix in range(BT // TCH):
                        # split each 2MB x-chunk across both hwdge queues
                        xs = xsp.tile([128, 8, TCH], f32r)
                        src = xT_d[:, tcix * TCH : (tcix + 1) * TCH].rearrange(
                            "(a p) t -> p a t", p=128
                        )
                        nc.sync.dma_start(xs[:, 0:4], src[:, 0:4])
                        nc.scalar.dma_start(xs[:, 4:8], src[:, 4:8])
                        if _rep == 0 and tcix == 0:
                            # later-needed weights queue behind the first
                            # x-chunk so PE starts ~5us after kernel entry
                            nc.sync.dma_start(
                                wk_s[:],
                                wk_d.rearrange("(a p) n -> p a n", p=128),
                            )
                            nc.sync.dma_start(
                                wv_s[:],
                                wv_d.rearrange("(a p) n -> p a n", p=128),
                            )
                            nc.scalar.dma_start(mask_f[:], mk_d[:])
                            nc.vector.tensor_copy(
                                out=mask_b[:], in_=mask_f[:]
                            )
                        if _rep == 0 and tcix == 1:
                            nc.scalar.dma_start(wo2_s[:], wo_d[:])
                        ps_q = qkvps.tile([128, TCH], f32, tag="pq")
                        ps_k = qkvps.tile([128, TCH], f32, tag="pk")
                        ps_v = qkvps.tile([128, TCH], f32, tag="pv")
                        for kk in range(8):
                            fl = dict(start=(kk == 0), stop=(kk == 7))
                            nc.tensor.matmul(
                                ps_q[:],
                                wq_s[:, kk],
                                xs[:, kk],
                                **fl,
                            )
                            nc.tensor.matmul(
                                ps_k[:],
                                wk_s[:, kk],
                                xs[:, kk],
                                **fl,
                            )
                            nc.tensor.matmul(
                                ps_v[:],
                                wv_s[:, kk],
                                xs[:, kk],
                                **fl,
                            )
                        sl = slice(tcix * TCH, (tcix + 1) * TCH)
                        # DVE is idle during stage A; ACT queue carries xs_hi DMA
                        nc.vector.tensor_copy(out=qT[:, sl], in_=ps_q[:])
                        nc.vector.tensor_copy(out=kT[:, sl], in_=ps_k[:])
                        nc.vector.tensor_copy(out=vT[:, sl], in_=ps_v[:])
                        for sub in range(TCH // 128):
                            st = tcix * 4 + sub
                            ps_t = trps.tile([128, 128], f32r)
                            nc.tensor.transpose(
                                ps_t[:], vT[:, st * 128 : (st + 1) * 128], ident[:]
                            )
                            nc.vector.tensor_copy(
                                out=v_ext[:, st, 0:64], in_=ps_t[:, 0:64]
                            )
                            nc.vector.tensor_copy(
                                out=v_ext[:, st, 129:193], in_=ps_t[:, 64:128]
                            )
                            nc.vector.tensor_copy(
                                out=v_ext[:, st, 64:129], in_=pat_f[:]
                            )

                # ---- stage B: attention + output projection ----
                # The two heads' K=64 score matmuls sit on disjoint PE row groups
                # (partition bases 0 and 64) and are issued back-to-back, so the
                # hardware runs them concurrently (row tiling).
                with (
                    tc.tile_pool(name="sps", bufs=3, space="PSUM") as sps,
                    tc.tile_pool(name="ops", bufs=2, space="PSUM") as ops,
                    tc.tile_pool(name="bcps", bufs=1, space="PSUM") as bcps,
                    tc.tile_pool(name="outps", bufs=2, space="PSUM") as outps,
                ):
                    for b in range(B):
                        boff = b * T
                        bst = b * NST
                        # both heads' normalized O^T stacked: rows 0:64 = head 0,
                        # 64:128 = head 1 -> K=128 output projection
                        oT2 = oTp.tile([128, T], f32r)
                        # denoms: head0's at partition 64, head1's at partition 0
                        # (lane-aligned with where each AV matmul drops them)
                        dn2 = dnp.tile([65, T], f32r)
                        for tj in range(NTCH):
                            tsl = slice(boff + tj * TCH, boff + (tj + 1) * TCH)
                            csl = slice(tj * TCH, (tj + 1) * TCH)
                            nsi = 4 * tj + 4
                            ps_os = [
                                ops.tile([128, TCH], f32, name="ps_o", tag="o")
                                for _ in range(HPC)
                            ]
                            for si in range(nsi):
                                r = si - 4 * tj
                                ssl = slice(boff + si * 128, boff + (si + 1) * 128)
                                # diagonal-crossing chunks only need cols
                                # >= 128*r: everything below is fully masked
                                c0 = 128 * r if r >= 1 else 0
                                nsl = slice(c0, TCH)
                                tnsl = slice(tsl.start + c0, tsl.stop)
                                ps_ss, ets = [], []
                                for h in range(HPC):
                                    hs = slice(h * 64, (h + 1) * 64)
                                    ps_s = sps.tile([128, TCH], f32, tag="s")
                                    ps_ss.append(ps_s)
                                    nc.tensor.matmul(
                                        ps_s[:, nsl],
                                        kT[hs, ssl],
                                        qT[hs, tnsl],
                                        start=True,
                                        stop=(r < 0),
                                    )
                                for h in range(HPC):
                                    if r >= 0:
                                        # add the [128,128] causal triangle
                                        # (0 / -1e30) over the ragged block
                                        # at cols c0:c0+128 (bf16: N=128 at
                                        # full rate, exact for 0/-1e30)
                                        nc.tensor.matmul(
                                            ps_ss[h][:, c0 : c0 + 128],
                                            ident_b[:],
                                            mask_b[:],
                                            start=False,
                                            stop=True,
                                        )
                                for h in range(HPC):
                                    et = etp.tile([128, TCH], f32r)
                                    ets.append(et)
                                    nc.scalar.activation(
                                        et[:, nsl], ps_ss[h][:, nsl], Exp,
                                        scale=0.125,
                                    )
                                avl = [
                                    v_ext[:, bst + si, 0:65],
                                    v_ext[:, bst + si, 65:193],
                                ]
                                for h in range(HPC):
                                    nc.tensor.matmul(
                                        ps_os[h][0 : avl[h].shape[-1], nsl],
                                        avl[h],
                                        ets[h][:, nsl],
                                        start=(si == 0),
                                        stop=(si == nsi - 1),
                                    )
                            # all four copies are partition-aligned (lane-safe)
                            nc.vector.tensor_copy(
                                out=oT2[0:64, csl], in_=ps_os[0][0:64, :]
                            )
                            nc.vector.tensor_copy(
                                out=dn2[64:65, csl], in_=ps_os[0][64:65, :]
                            )
                            nc.vector.tensor_copy(
                                out=oT2[64:128, csl], in_=ps_os[1][64:128, :]
                            )
                            nc.vector.tensor_copy(
                                out=dn2[0:1, csl], in_=ps_os[1][0:1, :]
                            )
                        # normalize: PE-broadcast each head's denom row onto its
                        # own 64 output partitions (h1 via col group 64 of the
                        # array), then one reciprocal + one multiply per chunk
                        for tj in range(NTCH):
                            csl = slice(tj * TCH, (tj + 1) * TCH)
                            ps_b = bcps.tile([128, TCH], f32)
                            nc.tensor.matmul(
                                ps_b[:],
                                sel_t[64:65, :],
                                dn2[64:65, csl],
                                start=True,
                                stop=False,
                            )
                            nc.tensor.matmul(
                                ps_b[:],
                                sel_t[0:1, :],
                                dn2[0:1, csl],
                                start=False,
                                stop=True,
                            )
                            rb = rbp.tile([128, TCH], f32)
                            nc.vector.reciprocal(rb[:], ps_b[:])
                            nc.vector.tensor_tensor(
                                out=oT2[:, csl],
                                in0=oT2[:, csl],
                                in1=rb[:],
                                op=mult,
                            )
                        # output projection: both heads in one K=128 contraction
                        for tt in range(T // 128):
                            stg = ostgp.tile([128, D], f32)
                            for mc in range(2):
                                msl = slice(mc * 512, (mc + 1) * 512)
                                ps_out = outps.tile([128, 512], f32)
                                nc.tensor.matmul(
                                    ps_out[:],
                                    oT2[:, tt * 128 : (tt + 1) * 128],
                                    wo2_s[:, msl],
                                    start=True,
                                    stop=True,
                                )
                                nc.any.tensor_copy(out=stg[:, msl], in_=ps_out[:])
                            # sync queue is idle after stage A; keep ACT for exp
                            nc.sync.dma_start(
                                out=out_d[boff + tt * 128 : boff + (tt + 1) * 128, :],
                                in_=stg[:],
                            )
    _split_matmul_waits()
    return nc


def _masks_np():
    """[128,128] additive causal triangle: 0 where j>=sp, else -1e30."""
    sp = np.arange(128)[:, None]
    j = np.arange(128)[None, :]
    return np.where(j >= sp, 0.0, -1e30).astype(np.float32)


def kernel(x, wq, wk, wv, wo):
    from concourse.bass_utils import run_bass_kernel_spmd

    if "nc" not in _cache:
        _cache["nc"] = _build()
    nc = _cache["nc"]

    xT = np.ascontiguousarray(
        np.asarray(x, dtype=np.float32).reshape(BT, D).T
    )
    wq = np.asarray(wq, dtype=np.float32)
    wk = np.asarray(wk, dtype=np.float32)
    wv = np.asarray(wv, dtype=np.float32)
    wo = np.asarray(wo, dtype=np.float32)
    masks = _masks_np()

    in_maps = []
    for c in range(NCORES):
        cs = slice(c * DC, (c + 1) * DC)
        in_maps.append(
            {
                "xT": xT,
                "wq": np.ascontiguousarray(wq[:, cs]),
                "wk": np.ascontiguousarray(wk[:, cs]),
                "wv": np.ascontiguousarray(wv[:, cs]),
                "wo": np.ascontiguousarray(wo[cs, :]),
                "masks": masks,
            }
        )

    res = run_bass_kernel_spmd(
        nc, in_maps, core_ids=list(range(NCORES)), **_cache.get("run_kwargs", {})
    )
    _cache["last_res"] = res
    acc = res.results[0]["out"].astype(np.float32)
    for c in range(1, NCORES):
        acc = acc + res.results[c]["out"]
    return acc.reshape(B, T, D)



# revision 12
# speedup vs baseline: 1.5745x; 1.5745x over previous
"""Causal self-attention kernel for 8 Trainium2 NeuronCores.

Problem: B=2, T=2048, D=1024, 16 heads x 64. Tensor-parallel over heads:
core c owns heads [2c, 2c+1] (128 projection columns), computes its partial
output through wo's matching 128 rows; host sums the 8 partials (TP
all-reduce done at unshard time).

All compute in bf16 (fp32 PSUM accumulation): rel-err ~6e-3 vs the fp32
reference, and bf16 halves SBUF streaming, weight-load time (FWL), DVE
copy cost and HBM traffic vs the fp32r variant, which ran throttled
(PE K=4/8) for 260us of a 342us kernel.

Device-side dataflow per core (no on-device transposes of activations or
probabilities):
  - x pre-transposed on host to xT [D, B*T] bf16; weights pre-arranged on
    host to [p, a, n] k-tile layout so every DMA line is 2KB.
  - Q^T/K^T/V^T projections: stationary = weight k-tile [128,128], moving
    = xT chunk; q/k/v share one 3-bank PSUM tile [128,1536] -> single DVE
    cast per 512-chunk into qkv_T [128, 3, BT] bf16.
  - V^T is transposed 128x128-wise on the PE into V_ext tiles [s=128,193]
    whose cols 64/65 are ones: head0 AV lhsT = cols 0:65 (denom at psum
    part 64), head1 AV lhsT = cols 65:193 (denom at part 0, O at 64:128).
    The ones/zeros middle is memset once up front.
  - scores: both heads share one [128, 2, 512] PSUM tile; the two K=64
    matmuls sit on disjoint PE row groups (partition bases 0/64), issued
    back-to-back -> hardware runs them concurrently. Causal mask added as
    a bf16 identity@mask matmul on the 128-col diagonal block only; fully
    masked chunks skipped.
  - ONE exp per s-tile over both heads' live region (strided AP
    [128,2,512-c0]) on the scalar engine, output bf16 -> halves ACT
    instruction count vs per-head exp.
  - AV: stationary = V_ext s-tile, moving = E chunk -> accumulate
    O^T_ext over s-tiles in PSUM. Head0 copy keeps rows 0:65 (O + denom),
    head1 copy keeps the full tile (denom row 0, O rows 64:128).
  - normalize: PE-broadcast (K=1 selector matmuls) spreads each head's
    denom row onto its 64 output partitions; reciprocal_approx_fast
    (single-pass custom DVE op, ~5x faster than reciprocal) then one DVE
    multiply per head.
  - out projection: the two heads' O live in different tiles (A rows
    0:64, B rows 64:128) -> two K=64 matmuls on disjoint row groups
    accumulate into one PSUM bank, running concurrently (row tiling).
    Staging copies alternate DVE/ACT; output DMA'd as bf16.
"""

import sys

if "/opt/trn_rl_repo" not in sys.path:
    sys.path.insert(0, "/opt/trn_rl_repo")

import numpy as np
import ml_dtypes

BF = ml_dtypes.bfloat16

N_HEAD = 16
D_HEAD = 64
D = 1024
B = 2
T = 2048
NCORES = 8
HPC = N_HEAD // NCORES          # heads per core = 2
DC = HPC * D_HEAD               # projection cols per core = 128
BT = B * T                      # 4096
TCH = 512                       # t-chunk (PSUM bank = 512 fp32)
XCH = 1024                      # x DMA chunk (2KB bf16 lines)
NTCH = T // TCH                 # 4 t-chunks per batch
NST = T // 128                  # 16 s-tiles per batch

_cache = {}


def _build():
    import concourse.bass as bass
    import concourse.mybir as mybir
    import concourse.tile as tile
    from concourse.masks import make_identity

    f32 = mybir.dt.float32
    bf16 = mybir.dt.bfloat16
    Exp = mybir.ActivationFunctionType.Exp
    mult = mybir.AluOpType.mult

    nc = bass.Bass("TRN2", target_bir_lowering=False, debug=False)

    xT_d = nc.dram_tensor("xT", [D, BT], bf16, kind="ExternalInput").ap()
    wq_d = nc.dram_tensor("wq", [128, 8, 128], bf16, kind="ExternalInput").ap()
    wk_d = nc.dram_tensor("wk", [128, 8, 128], bf16, kind="ExternalInput").ap()
    wv_d = nc.dram_tensor("wv", [128, 8, 128], bf16, kind="ExternalInput").ap()
    wo_d = nc.dram_tensor("wo", [DC, D], bf16, kind="ExternalInput").ap()
    mk_d = nc.dram_tensor("masks", [128, 128], bf16, kind="ExternalInput").ap()
    out_d = nc.dram_tensor("out", [BT, D], bf16, kind="ExternalOutput").ap()

    def _split_matmul_waits():
        """walrus codegen allows a single sync-wait on several instruction
        encodings (self-loading matmul LW structs, DMA triggers); hoist
        extra waits onto same-engine NoOps placed just before."""
        for bb in nc.m.functions[0].blocks:
            out = []
            for ins in bb.instructions:
                si = getattr(ins, "sync_info", None)
                if si and len(si.on_wait) > 1:
                    for i, w in enumerate(list(si.on_wait[:-1])):
                        nop = mybir.InstNoOp(
                            name=f"{ins.name}-ws{i}",
                            engine=ins.engine,
                            sync_info=mybir.SyncInfo(on_wait=[w], on_update=[]),
                        )
                        nc.register_instruction(nop)
                        out.append(nop)
                    si.on_wait = [si.on_wait[-1]]
                out.append(ins)
            bb.instructions = out

    with tile.TileContext(nc) as tc:
        with (
            tc.tile_pool(name="const", bufs=1) as constp,
            tc.tile_pool(name="wpool", bufs=1) as wpool,
            tc.tile_pool(name="resid", bufs=1) as resid,
            tc.tile_pool(name="xs", bufs=2) as xsp,
            tc.tile_pool(name="et", bufs=4) as etp,
            tc.tile_pool(name="oT", bufs=2) as oTp,
            tc.tile_pool(name="dn", bufs=2) as dnp,
            tc.tile_pool(name="rb", bufs=2) as rbp,
            tc.tile_pool(name="ostg", bufs=3) as ostgp,
        ):
            ident_f = constp.tile([128, 128], f32)
            make_identity(nc, ident_f[:])
            ident_b = constp.tile([128, 128], bf16)
            nc.vector.tensor_copy(out=ident_b[:], in_=ident_f[:])
            mask_b = constp.tile([128, 128], bf16)
            # denom-broadcast selector rows (lane-matched to the AV psum
            # layout): row 64 (head0's denom partition) spreads onto
            # parts 0:64, row 0 (head1's) onto parts 64:128.
            sel_t = constp.tile([65, 128], bf16)
            nc.vector.memset(sel_t[:], 0.0)
            nc.vector.memset(sel_t[64:65, 0:64], 1.0)
            nc.vector.memset(sel_t[0:1, 64:128], 1.0)

            wq_s = wpool.tile([128, 8, 128], bf16, tag="wq")
            wk_s = wpool.tile([128, 8, 128], bf16, tag="wk")
            wv_s = wpool.tile([128, 8, 128], bf16, tag="wv")
            # qkv weights on the sync hwdge queue (critical path to first
            # matmul); masks/wo on the scalar queue.
            nc.sync.dma_start(wq_s[:], wq_d[:])
            wo2_s = wpool.tile([DC, D], bf16, tag="wo")

            # qkv_T[:, 0]=Q^T, [:,1]=K^T, [:,2]=V^T, each [128, BT]
            qkv_T = resid.tile([128, 3, BT], bf16, tag="qkvT")
            qT = qkv_T[:, 0]
            kT = qkv_T[:, 1]
            vT = qkv_T[:, 2]
            # Per s-tile layout (width 193):
            #   cols   0:64  V_h0            -> head0 AV lhsT = cols 0:65
            #   col      64  ones (h0 denom)    (O_h0 at psum parts 0:64,
            #   col      65  ones (h1 denom)     denom at part 64)
            #   cols 66:129  zeros           -> head1 AV lhsT = cols 65:193
            #   cols 129:193 V_h1               (denom at psum part 0, O_h1
            #                                    at psum parts 64:128)
            v_ext = resid.tile([128, 2 * NST, 193], bf16, tag="vext")
            # ones/zeros middle built once: strided memsets over all s-tiles
            nc.vector.memset(v_ext[:, :, 64:129], 0.0)
            nc.vector.memset(v_ext[:, :, 64:66], 1.0)

            # ---- stage A: QKV^T projections over 4 x-chunks of 1024,
            # with V 128x128 PE-transposes fused in ----
            with (
                tc.tile_pool(name="qkvps", bufs=2, space="PSUM") as qkvps,
                tc.tile_pool(name="trps", bufs=2, space="PSUM") as trps,
            ):
                for tcix in range(BT // XCH):
                    # split each 2MB x-chunk across both hwdge queues
                    xs = xsp.tile([128, 8, XCH], bf16)
                    src = xT_d[:, tcix * XCH : (tcix + 1) * XCH].rearrange(
                        "(a p) t -> p a t", p=128
                    )
                    nc.sync.dma_start(xs[:, 0:4], src[:, 0:4])
                    nc.scalar.dma_start(xs[:, 4:8], src[:, 4:8])
                    if tcix == 0:
                        # later-needed weights queue behind the first
                        # x-chunk so PE starts as early as possible
                        nc.sync.dma_start(wk_s[:], wk_d[:])
                        nc.sync.dma_start(wv_s[:], wv_d[:])
                        nc.scalar.dma_start(mask_b[:], mk_d[:])
                    if tcix == 1:
                        nc.scalar.dma_start(wo2_s[:], wo_d[:])
                    for sub in range(XCH // TCH):
                        coff = tcix * XCH + sub * TCH
                        xsl = slice(sub * TCH, (sub + 1) * TCH)
                        ps_qkv = qkvps.tile([128, 3, TCH], f32, tag="pqkv")
                        for kk in range(8):
                            fl = dict(start=(kk == 0), stop=(kk == 7))
                            nc.tensor.matmul(
                                ps_qkv[:, 0], wq_s[:, kk], xs[:, kk, xsl], **fl
                            )
                            nc.tensor.matmul(
                                ps_qkv[:, 1], wk_s[:, kk], xs[:, kk, xsl], **fl
                            )
                            nc.tensor.matmul(
                                ps_qkv[:, 2], wv_s[:, kk], xs[:, kk, xsl], **fl
                            )
                        # single 3-bank cast; ACT takes the v_ext copies
                        nc.vector.tensor_copy(
                            out=qkv_T[:, :, coff : coff + TCH], in_=ps_qkv[:]
                        )
                        for stsub in range(TCH // 128):
                            st = coff // 128 + stsub
                            ps_t = trps.tile([128, 128], bf16)
                            nc.tensor.transpose(
                                ps_t[:],
                                vT[:, st * 128 : (st + 1) * 128],
                                ident_b[:],
                            )
                            nc.scalar.copy(
                                out=v_ext[:, st, 0:64], in_=ps_t[:, 0:64]
                            )
                            nc.scalar.copy(
                                out=v_ext[:, st, 129:193], in_=ps_t[:, 64:128]
                            )

            # ---- stage B: attention + output projection ----
            for b in range(B):
                boff = b * T
                bst = b * NST
                # both heads' normalized O^T stacked: rows 0:64 = head 0,
                # 64:128 = head 1 -> K=128 output projection
                oT2 = oTp.tile([128, T], bf16, tag="oT2")
                # denoms: head0's at partition 64, head1's at partition 0
                # (lane-aligned with where each AV matmul drops them).
                # Rows 1:64 feed the K=65 broadcast matmul as zeros
                # (garbage would poison it: NaN*0=NaN).
                dn2 = dnp.tile([65, T], bf16)
                nc.vector.memset(dn2[0:64, :], 0.0)
                with (
                    tc.tile_pool(name="sps", bufs=2, space="PSUM") as sps,
                    tc.tile_pool(name="ops", bufs=2, space="PSUM") as ops,
                ):
                    for tj in range(NTCH):
                        tsl0 = boff + tj * TCH
                        csl = slice(tj * TCH, (tj + 1) * TCH)
                        nsi = 4 * tj + 4
                        ps_o2 = ops.tile([128, HPC, TCH], f32, tag="o")
                        for si in range(nsi):
                            r = si - 4 * tj
                            ssl = slice(boff + si * 128, boff + (si + 1) * 128)
                            # diagonal-crossing chunks only need cols
                            # >= 128*r: everything below is fully masked
                            c0 = 128 * r if r >= 1 else 0
                            nsl = slice(c0, TCH)
                            tnsl = slice(tsl0 + c0, tsl0 + TCH)
                            # both heads' scores share one 2-bank psum
                            # tile; K=64 matmuls on disjoint row groups
                            # run concurrently
                            ps_s = sps.tile([128, HPC, TCH], f32, tag="s")
                            for h in range(HPC):
                                hs = slice(h * 64, (h + 1) * 64)
                                nc.tensor.matmul(
                                    ps_s[:, h, nsl],
                                    kT[hs, ssl],
                                    qT[hs, tnsl],
                                    start=True,
                                    stop=(r < 0),
                                )
                            if r >= 0:
                                # add the [128,128] causal triangle
                                # (0 / -1e30) over the ragged block at
                                # cols c0:c0+128 of each head
                                for h in range(HPC):
                                    nc.tensor.matmul(
                                        ps_s[:, h, c0 : c0 + 128],
                                        ident_b[:],
                                        mask_b[:],
                                        start=False,
                                        stop=True,
                                    )
                            # one exp over both heads' live region
                            et = etp.tile([128, HPC, TCH], bf16)
                            nc.scalar.activation(
                                et[:, :, nsl], ps_s[:, :, nsl], Exp,
                                scale=0.125,
                            )
                            avl = [
                                v_ext[:, bst + si, 0:65],
                                v_ext[:, bst + si, 65:193],
                            ]
                            for h in range(HPC):
                                nc.tensor.matmul(
                                    ps_o2[0 : avl[h].shape[-1], h, nsl],
                                    avl[h],
                                    et[:, h, nsl],
                                    start=(si == 0),
                                    stop=(si == nsi - 1),
                                )
                        # partition-aligned casts: O rows on DVE, the two
                        # denom rows on ACT
                        nc.vector.tensor_copy(
                            out=oT2[0:64, csl], in_=ps_o2[0:64, 0]
                        )
                        nc.vector.tensor_copy(
                            out=oT2[64:128, csl], in_=ps_o2[64:128, 1]
                        )
                        nc.scalar.copy(
                            out=dn2[64:65, csl], in_=ps_o2[64:65, 0]
                        )
                        nc.scalar.copy(
                            out=dn2[0:1, csl], in_=ps_o2[0:1, 1]
                        )
                # normalize + output projection (psum pools re-opened:
                # scores/AV banks are free now)
                with (
                    tc.tile_pool(name="bcps", bufs=2, space="PSUM") as bcps,
                    tc.tile_pool(name="outps", bufs=4, space="PSUM") as outps,
                ):
                    for tj in range(NTCH):
                        csl = slice(tj * TCH, (tj + 1) * TCH)
                        ps_b = bcps.tile([128, TCH], f32)
                        # PE-broadcast each head's denom row onto its own
                        # 64 output partitions. ONE K=65 matmul: two
                        # accumulating K=1 matmuls with different row
                        # bases drain the same bank concurrently (disjoint
                        # row groups) -> fatal PSUM collision on HW.
                        nc.tensor.matmul(
                            ps_b[:],
                            sel_t[:, :],
                            dn2[:, csl],
                            start=True,
                            stop=True,
                        )
                        rb = rbp.tile([128, TCH], f32)
                        nc.vector.reciprocal_approx_fast(rb[:], ps_b[:])
                        nc.vector.tensor_tensor(
                            out=oT2[:, csl],
                            in0=oT2[:, csl],
                            in1=rb[:],
                            op=mult,
                        )
                    # output projection: both heads in one K=128
                    # contraction
                    for tt in range(T // 128):
                        stg = ostgp.tile([128, D], bf16)
                        tts = slice(tt * 128, (tt + 1) * 128)
                        for mc in range(2):
                            msl = slice(mc * 512, (mc + 1) * 512)
                            ps_out = outps.tile([128, 512], f32)
                            nc.tensor.matmul(
                                ps_out[:],
                                oT2[:, tts],
                                wo2_s[:, msl],
                                start=True,
                                stop=True,
                            )
                            # alternate the staging casts DVE/ACT
                            if mc == 0:
                                nc.vector.tensor_copy(
                                    out=stg[:, msl], in_=ps_out[:]
                                )
                            else:
                                nc.scalar.copy(out=stg[:, msl], in_=ps_out[:])
                        # sync queue is idle after stage A
                        nc.sync.dma_start(
                            out=out_d[boff + tt * 128 : boff + (tt + 1) * 128, :],
                            in_=stg[:],
                        )
    _split_matmul_waits()
    # custom-DVE ops (reciprocal_approx_fast) are extended-inst InstISA
    # subclasses whose .instr bytes are populated by this pass; without it
    # walrus codegen fails with "ISA wrong length".
    from concourse.library_overlay import lower_extended_insts

    lower_extended_insts(nc)
    return nc


def _masks_np():
    """[128,128] additive causal triangle: 0 where j>=sp, else -1e30."""
    sp = np.arange(128)[:, None]
    j = np.arange(128)[None, :]
    return np.where(j >= sp, 0.0, -1e30).astype(BF)


def kernel(x, wq, wk, wv, wo):
    from concourse.bass_utils import run_bass_kernel_spmd

    if "nc" not in _cache:
        _cache["nc"] = _build()
    nc = _cache["nc"]

    xT = np.ascontiguousarray(
        np.asarray(x, dtype=np.float32).reshape(BT, D).T
    ).astype(BF)
    wq = np.asarray(wq, dtype=np.float32)
    wk = np.asarray(wk, dtype=np.float32)
    wv = np.asarray(wv, dtype=np.float32)
    wo = np.asarray(wo, dtype=np.float32)
    masks = _masks_np()

    def _ktiles(w, cs):
        # [1024, 128] col-slice -> [p, a, n] k-tile layout, contiguous
        return np.ascontiguousarray(
            w[:, cs].reshape(8, 128, DC).transpose(1, 0, 2)
        ).astype(BF)

    in_maps = []
    for c in range(NCORES):
        cs = slice(c * DC, (c + 1) * DC)
        in_maps.append(
            {
                "xT": xT,
                "wq": _ktiles(wq, cs),
                "wk": _ktiles(wk, cs),
                "wv": _ktiles(wv, cs),
                "wo": np.ascontiguousarray(wo[cs, :]).astype(BF),
                "masks": masks,
            }
        )

    res = run_bass_kernel_spmd(
        nc, in_maps, core_ids=list(range(NCORES)), **_cache.get("run_kwargs", {})
    )
    _cache["last_res"] = res
    acc = res.results[0]["out"].astype(np.float32)
    for c in range(1, NCORES):
        acc = acc + res.results[c]["out"].astype(np.float32)
    return acc.reshape(B, T, D)
